# revision 7
# baseline (speedup 1.0000x reference)
"""GRU cell kernel for Trainium2, 8-core data-parallel, single dispatch.

Strategy
--------
Data-parallel on batch across 8 cores; each core processes its full
2048-row shard in ONE NEFF dispatch.  All on-chip compute happens in
*transposed space* ([hidden, batch]) so every matmul contraction lands
on SBUF partitions with no on-device transposes:

    r^T = sigmoid(W_r @ x^T + U_r @ h^T + b_r)
    u^T = sigmoid(W_u @ x^T + U_u @ h^T + b_u)
    c^T = tanh   (W   @ x^T + U  @ (h.r)^T + b_c)
    o^T = h^T + u^T * (c^T - h^T)

Matmuls run in bf16.  The 2048-wide batch gives 4 moving slices of 512
per stationary weight tile, which keeps the tensor engine at its
~216 ns/MM back-to-back cadence (weight-swap drain stalls amortize).

DMA conveyor: the HWDGE rings dispatch ~1 DMA instruction per ~600 ns
regardless of size, so per-128x128-tile weight loads (384 of them)
rate-limit the whole kernel (measured: PE catches the weight stream
and idles 47 us).  v3 loads weights as 48 contiguous 256 KiB slabs
(one per (matrix, output-tile), host-side packed) on the Sync ring,
and puts x on the Scalar ring so the two conveyors run in parallel.
The R phase consumes k-tiles in arrival order.

SBUF budget (per partition, ~208 KiB usable): weights 96K + x 32K +
h 32K + hr 16K + pools ~20K.  hr is stored fp8_e4m3 (moving operand
only) and the output chain runs bf16 in-place in the c tile; measured
end-to-end rel err 7.5e-3 vs the 2e-2 budget.

DMA rules: descriptors encode exactly ONE sync wait, so no load DMA
may target a recycled tile slot (loads carry only queue-FIFO waits ->
every DMA'd tile gets its own slot), and the 8 output stores ride 8
distinct SWDGE queues so their single RAW wait fits.
"""

import sys

sys.path.insert(0, "/opt/trn_rl_repo")

import numpy as np
import ml_dtypes
from contextlib import ExitStack

import concourse.bass as bass
import concourse.bacc as bacc
import concourse.mybir as mybir
from concourse import tile
from concourse.bass_utils import run_bass_kernel_spmd

BF16 = mybir.dt.bfloat16
FP8 = mybir.dt.float8e4
F32 = mybir.dt.float32
AF = mybir.ActivationFunctionType

N_CORES = 8
B = 16384
D = 1024  # IN == H
B_SHARD = B // N_CORES  # 2048 rows per core, single dispatch
BW = 512  # matmul moving width (one fp32 PSUM bank)

# k-tile consumption order: ascending matches the conveyor arrival order
KORD = list(range(8))


def build_nc(d=D, b_shard=B_SHARD, bw=BW):
    """Build the SPMD per-core Bass program.

    Packed weight order: 0=W_r, 1=U_r, 2=W_u, 3=U_u, 4=W, 5=U.
    Bias columns: [r: 0..nh) [u: nh..2nh) [c: 2nh..3nh).
    """
    nk = d // 128
    nh = d // 128
    nb = b_shard // bw

    nc = bacc.Bacc("TRN2", target_bir_lowering=False)
    xt = nc.dram_tensor("xt", [d, b_shard], BF16, kind="ExternalInput")
    ht = nc.dram_tensor("ht", [d, b_shard], BF16, kind="ExternalInput")
    # weight slab (mat, j): [128 partitions, nk*128] contiguous
    wts = nc.dram_tensor("wts", [6, nh, 128, nk * 128], BF16, kind="ExternalInput")
    bias = nc.dram_tensor("bias", [128, 3 * nh], F32, kind="ExternalInput")
    out = nc.dram_tensor("out", [d, b_shard], BF16, kind="ExternalOutput")

    with tile.TileContext(nc) as tc, ExitStack() as ctx:
        xp = ctx.enter_context(tc.tile_pool(name="xp", bufs=nk))
        hp = ctx.enter_context(tc.tile_pool(name="hp", bufs=nk))
        hrp = ctx.enter_context(tc.tile_pool(name="hrp", bufs=nh))
        rp = ctx.enter_context(tc.tile_pool(name="rp", bufs=2))
        up = ctx.enter_context(tc.tile_pool(name="up", bufs=2))
        cp = ctx.enter_context(tc.tile_pool(name="cp", bufs=3))
        # every weight slab gets its own slot: no DMA slot reuse anywhere
        wp = ctx.enter_context(tc.tile_pool(name="wp", bufs=6 * nh))
        bp = ctx.enter_context(tc.tile_pool(name="bp", bufs=1))
        pp = ctx.enter_context(tc.tile_pool(name="pp", bufs=8, space="PSUM"))

        wslabs = {}

        def load_w(mat, j):
            t = wp.tile([128, nk * 128], BF16, name="wslab")
            nc.sync.dma_start(t, wts[mat, j, :, :])
            wslabs[(mat, j)] = t

        xts, hts = [None] * nk, [None] * nk
        half = b_shard // 2

        # Half-tile loads: the conveyor is HBM-fair-share bound at startup,
        # so finer DMA granularity gets the first k-tiles consumable sooner.
        def load_x(k):
            xts[k] = xp.tile([128, b_shard], BF16, name="xtile")
            nc.scalar.dma_start(xts[k][:, :half], xt[k * 128 : (k + 1) * 128, :half])
            nc.scalar.dma_start(xts[k][:, half:], xt[k * 128 : (k + 1) * 128, half:])

        def load_h(k):
            hts[k] = hp.tile([128, b_shard], BF16, name="htile")
            nc.sync.dma_start(hts[k][:, :half], ht[k * 128 : (k + 1) * 128, :half])
            nc.sync.dma_start(hts[k][:, half:], ht[k * 128 : (k + 1) * 128, half:])

        # The startup burst shares HBM across ~9 DMA lanes, so the very
        # first tiles (gate 0's stationary + x0/h0) load in small chunks
        # to land as early as possible.
        def load_x0():
            xts[0] = xp.tile([128, b_shard], BF16, name="xtile")
            for q in range(4):
                s = slice(q * bw, (q + 1) * bw)
                nc.scalar.dma_start(xts[0][:, s], xt[0:128, s])

        def load_w0(mat):
            t = wp.tile([128, nk * 128], BF16, name="wslab")
            for q in range(4):
                s = slice(q * 256, (q + 1) * 256)
                nc.sync.dma_start(t[:, s], wts[mat, 0, :, s])
            wslabs[(mat, 0)] = t

        def load_h0():
            hts[0] = hp.tile([128, b_shard], BF16, name="htile")
            for q in range(4):
                s = slice(q * bw, (q + 1) * bw)
                nc.sync.dma_start(hts[0][:, s], ht[0:128, s])

        # scalar ring: all of x, then bias.  sync ring: gate-0 R slabs, h
        # interleaved with the later R slabs, then UC slabs in use order.
        load_x0()
        for k in range(1, nk):
            load_x(k)
        btile = bp.tile([128, 3 * nh], F32, name="btile")
        nc.scalar.dma_start(btile, bias[:, :])
        load_w0(0)
        load_w0(1)
        load_h0()
        load_h(1)
        load_w(0, 1)
        load_w(1, 1)
        load_h(2)
        load_h(3)
        load_w(0, 2)
        load_w(1, 2)
        load_h(4)
        load_h(5)
        load_w(0, 3)
        load_w(1, 3)
        load_h(6)
        load_h(7)
        for j in range(4, nh):
            load_w(0, j)
            load_w(1, j)
        for j in range(nh):
            for mat in (2, 3, 4, 5):
                load_w(mat, j)

        def gate_matmuls(j, mat_x, mov_x, mat_h, mov_h):
            """Accumulate x-part + h-part for gate tile j into nb PSUM banks."""
            ps = [pp.tile([128, bw], F32, name="ps") for _ in range(nb)]
            for mi, (mat, mov) in enumerate(((mat_x, mov_x), (mat_h, mov_h))):
                slab = wslabs[(mat, j)]
                for ki, k in enumerate(KORD):
                    lhsT = slab[:, k * 128 : (k + 1) * 128]
                    for b in range(nb):
                        nc.tensor.matmul(
                            ps[b],
                            lhsT,
                            mov[k][:, b * bw : (b + 1) * bw],
                            start=(mi == 0 and ki == 0),
                            stop=(mi == 1 and ki == nk - 1),
                        )
            return ps

        # R phase: r = sigmoid(...); hr = h * r in fp8 (feeds the c matmuls)
        hrs = []
        for j in range(nh):
            ps = gate_matmuls(j, 0, xts, 1, hts)
            rtile = rp.tile([128, b_shard], BF16, name="rtile")
            for b in range(nb):
                nc.scalar.activation(
                    rtile[:, b * bw : (b + 1) * bw], ps[b], AF.Sigmoid,
                    bias=btile[:, j : j + 1],
                )
            hrtile = hrp.tile([128, b_shard], FP8, name="hrtile")
            nc.vector.tensor_mul(hrtile, hts[j], rtile)
            hrs.append(hrtile)

        # U+C fused per j, out chain in-place in ctile, chunked per bank so
        # the tail (ACT -> DVE -> store) pipelines at 512 granularity.
        for j in range(nh):
            ps = gate_matmuls(j, 2, xts, 3, hts)
            util = up.tile([128, b_shard], BF16, name="utile")
            for b in range(nb):
                nc.scalar.activation(
                    util[:, b * bw : (b + 1) * bw], ps[b], AF.Sigmoid,
                    bias=btile[:, nh + j : nh + j + 1],
                )
            ps = gate_matmuls(j, 4, xts, 5, hrs)
            ctile = cp.tile([128, b_shard], BF16, name="ctile")
            for b in range(nb):
                s = slice(b * bw, (b + 1) * bw)
                nc.scalar.activation(
                    ctile[:, s], ps[b], AF.Tanh,
                    bias=btile[:, 2 * nh + j : 2 * nh + j + 1],
                )
                # o = h + u*(c - h), computed in place in ctile (bf16)
                nc.vector.tensor_sub(ctile[:, s], ctile[:, s], hts[j][:, s])
                nc.vector.tensor_mul(ctile[:, s], util[:, s], ctile[:, s])
                nc.vector.tensor_add(ctile[:, s], ctile[:, s], hts[j][:, s])
                # per-bank store on the sync HWDGE ring (idle after loads):
                # pipelines the tail instead of one big end-of-gate store.
                nc.sync.dma_start(out[j * 128 : (j + 1) * 128, s], ctile[:, s])

    # Bacc lowering: splits multi-wait sync into InstEventSemaphore ops
    # (hardware allows one wait per instruction), allocates registers, etc.
    nc.compile()
    return nc


def pack_inputs(inputs, d=D, b_shard=B_SHARD, n_shards=N_CORES):
    """Host-side shard + transpose + cast. Returns per-shard input maps."""
    nk = d // 128
    nh = d // 128
    x = np.asarray(inputs["x_t"], np.float32)
    h = np.asarray(inputs["h_prev"], np.float32)

    mats = [inputs["W_r"], inputs["U_r"], inputs["W_u"], inputs["U_u"],
            inputs["W"], inputs["U"]]
    wts = np.empty((6, nh, 128, nk * 128), ml_dtypes.bfloat16)
    for i, m in enumerate(mats):
        mt = np.asarray(m, np.float32).T.astype(ml_dtypes.bfloat16)  # [in, out]
        # wts[i, j, p, k*128+m] = M.T[k*128+p, j*128+m]
        wts[i] = mt.reshape(nk, 128, nh, 128).transpose(2, 1, 0, 3).reshape(
            nh, 128, nk * 128
        )

    b_r = np.asarray(inputs["b_Wr"], np.float32) + np.asarray(inputs["b_Ur"], np.float32)
    b_u = np.asarray(inputs["b_Wu"], np.float32) + np.asarray(inputs["b_Uu"], np.float32)
    b_c = np.asarray(inputs["b_W"], np.float32) + np.asarray(inputs["b_U"], np.float32)
    bias = np.concatenate(
        [bb.reshape(nh, 128).T for bb in (b_r, b_u, b_c)], axis=1
    ).astype(np.float32)  # [128, 3*nh]

    in_maps = []
    for s in range(n_shards):
        rows = slice(s * b_shard, (s + 1) * b_shard)
        xT = np.ascontiguousarray(x[rows].T).astype(ml_dtypes.bfloat16)
        hT = np.ascontiguousarray(h[rows].T).astype(ml_dtypes.bfloat16)
        in_maps.append({"xt": xT, "ht": hT, "wts": wts, "bias": bias})
    return in_maps


_NC_CACHE = {}


def _get_nc():
    if "nc" not in _NC_CACHE:
        _NC_CACHE["nc"] = build_nc()
    return _NC_CACHE["nc"]


def _run(inputs, **spmd_kwargs):
    nc = _get_nc()
    in_maps = pack_inputs(inputs)
    res = run_bass_kernel_spmd(nc, in_maps, list(range(N_CORES)), **spmd_kwargs)
    out = np.empty((B, D), np.float32)
    for c in range(N_CORES):
        out[c * B_SHARD : (c + 1) * B_SHARD, :] = (
            res.results[c]["out"].astype(np.float32).T
        )
    return out, [res]


def kernel(**inputs):
    out, _ = _run(inputs)
    return out


# revision 8
# speedup vs baseline: 1.0019x; 1.0019x over previous
"""GRU cell kernel for Trainium2, 8-core data-parallel, single dispatch.

Strategy
--------
Data-parallel on batch across 8 cores; each core processes its full
2048-row shard in ONE NEFF dispatch.  All on-chip compute happens in
*transposed space* ([hidden, batch]) so every matmul contraction lands
on SBUF partitions with no on-device transposes:

    r^T = sigmoid(W_r @ x^T + U_r @ h^T + b_r)
    u^T = sigmoid(W_u @ x^T + U_u @ h^T + b_u)
    c^T = tanh   (W   @ x^T + U  @ (h.r)^T + b_c)
    o^T = h^T + u^T * (c^T - h^T)

Matmuls run in bf16.  The 2048-wide batch gives 4 moving slices of 512
per stationary weight tile, which keeps the tensor engine at its
~216 ns/MM back-to-back cadence (weight-swap drain stalls amortize).

DMA conveyor: the HWDGE rings dispatch ~1 DMA instruction per ~600 ns
regardless of size, so per-128x128-tile weight loads (384 of them)
rate-limit the whole kernel (measured: PE catches the weight stream
and idles 47 us).  v3 loads weights as 48 contiguous 256 KiB slabs
(one per (matrix, output-tile), host-side packed) on the Sync ring,
and puts x on the Scalar ring so the two conveyors run in parallel.
The R phase consumes k-tiles in arrival order.

SBUF budget (per partition, ~208 KiB usable): weights 96K + x 32K +
h 32K + hr 16K + pools ~20K.  hr is stored fp8_e4m3 (moving operand
only) and the output chain runs bf16 in-place in the c tile; measured
end-to-end rel err 7.5e-3 vs the 2e-2 budget.

DMA rules: descriptors encode exactly ONE sync wait, so no load DMA
may target a recycled tile slot (loads carry only queue-FIFO waits ->
every DMA'd tile gets its own slot), and the 8 output stores ride 8
distinct SWDGE queues so their single RAW wait fits.
"""

import sys

sys.path.insert(0, "/opt/trn_rl_repo")

import numpy as np
import ml_dtypes
from contextlib import ExitStack

import concourse.bass as bass
import concourse.bacc as bacc
import concourse.mybir as mybir
from concourse import tile
from concourse.bass_utils import run_bass_kernel_spmd

BF16 = mybir.dt.bfloat16
FP8 = mybir.dt.float8e4
F32 = mybir.dt.float32
AF = mybir.ActivationFunctionType

N_CORES = 8
B = 16384
D = 1024  # IN == H
B_SHARD = B // N_CORES  # 2048 rows per core, single dispatch
BW = 512  # matmul moving width (one fp32 PSUM bank)

# k-tile consumption order: ascending matches the conveyor arrival order
KORD = list(range(8))


def build_nc(d=D, b_shard=B_SHARD, bw=BW):
    """Build the SPMD per-core Bass program.

    Packed weight order: 0=W_r, 1=U_r, 2=W_u, 3=U_u, 4=W, 5=U.
    Bias columns: [r: 0..nh) [u: nh..2nh) [c: 2nh..3nh).
    """
    nk = d // 128
    nh = d // 128
    nb = b_shard // bw

    nc = bacc.Bacc("TRN2", target_bir_lowering=False)
    xt = nc.dram_tensor("xt", [d, b_shard], BF16, kind="ExternalInput")
    ht = nc.dram_tensor("ht", [d, b_shard], BF16, kind="ExternalInput")
    # weight slab (mat, j): [128 partitions, nk*128] contiguous
    wts = nc.dram_tensor("wts", [6, nh, 128, nk * 128], BF16, kind="ExternalInput")
    bias = nc.dram_tensor("bias", [128, 3 * nh], F32, kind="ExternalInput")
    out = nc.dram_tensor("out", [d, b_shard], BF16, kind="ExternalOutput")

    with tile.TileContext(nc) as tc, ExitStack() as ctx:
        xp = ctx.enter_context(tc.tile_pool(name="xp", bufs=nk))
        hp = ctx.enter_context(tc.tile_pool(name="hp", bufs=nk))
        hrp = ctx.enter_context(tc.tile_pool(name="hrp", bufs=nh))
        rp = ctx.enter_context(tc.tile_pool(name="rp", bufs=2))
        up = ctx.enter_context(tc.tile_pool(name="up", bufs=2))
        cp = ctx.enter_context(tc.tile_pool(name="cp", bufs=3))
        # every weight slab gets its own slot: no DMA slot reuse anywhere
        wp = ctx.enter_context(tc.tile_pool(name="wp", bufs=6 * nh))
        bp = ctx.enter_context(tc.tile_pool(name="bp", bufs=1))
        pp = ctx.enter_context(tc.tile_pool(name="pp", bufs=8, space="PSUM"))

        wslabs = {}

        def load_w(mat, j):
            t = wp.tile([128, nk * 128], BF16, name="wslab")
            nc.sync.dma_start(t, wts[mat, j, :, :])
            wslabs[(mat, j)] = t

        xts, hts = [None] * nk, [None] * nk
        half = b_shard // 2

        # Half-tile loads: the conveyor is HBM-fair-share bound at startup,
        # so finer DMA granularity gets the first k-tiles consumable sooner.
        def load_x(k):
            xts[k] = xp.tile([128, b_shard], BF16, name="xtile")
            nc.scalar.dma_start(xts[k][:, :half], xt[k * 128 : (k + 1) * 128, :half])
            nc.scalar.dma_start(xts[k][:, half:], xt[k * 128 : (k + 1) * 128, half:])

        def load_h(k):
            hts[k] = hp.tile([128, b_shard], BF16, name="htile")
            nc.sync.dma_start(hts[k][:, :half], ht[k * 128 : (k + 1) * 128, :half])
            nc.sync.dma_start(hts[k][:, half:], ht[k * 128 : (k + 1) * 128, half:])

        # scalar ring: all of x, then bias.  sync ring: gate-0 R slabs, h
        # interleaved with the later R slabs, then UC slabs in use order.
        # (The startup is HBM-bound either way: finer first-tile chunking
        # moves the first MM earlier but the stall total is unchanged.)
        for k in range(nk):
            load_x(k)
        btile = bp.tile([128, 3 * nh], F32, name="btile")
        nc.scalar.dma_start(btile, bias[:, :])
        load_w(0, 0)
        load_w(1, 0)
        load_h(0)
        load_h(1)
        load_w(0, 1)
        load_w(1, 1)
        load_h(2)
        load_h(3)
        load_w(0, 2)
        load_w(1, 2)
        load_h(4)
        load_h(5)
        load_w(0, 3)
        load_w(1, 3)
        load_h(6)
        load_h(7)
        for j in range(4, nh):
            load_w(0, j)
            load_w(1, j)
        for j in range(nh):
            for mat in (2, 3, 4, 5):
                load_w(mat, j)

        def gate_matmuls(j, mat_x, mov_x, mat_h, mov_h):
            """Accumulate x-part + h-part for gate tile j into nb PSUM banks."""
            ps = [pp.tile([128, bw], F32, name="ps") for _ in range(nb)]
            for mi, (mat, mov) in enumerate(((mat_x, mov_x), (mat_h, mov_h))):
                slab = wslabs[(mat, j)]
                for ki, k in enumerate(KORD):
                    lhsT = slab[:, k * 128 : (k + 1) * 128]
                    for b in range(nb):
                        nc.tensor.matmul(
                            ps[b],
                            lhsT,
                            mov[k][:, b * bw : (b + 1) * bw],
                            start=(mi == 0 and ki == 0),
                            stop=(mi == 1 and ki == nk - 1),
                        )
            return ps

        # R phase: r = sigmoid(...); hr = h * r in fp8 (feeds the c matmuls)
        hrs = []
        for j in range(nh):
            ps = gate_matmuls(j, 0, xts, 1, hts)
            rtile = rp.tile([128, b_shard], BF16, name="rtile")
            for b in range(nb):
                nc.scalar.activation(
                    rtile[:, b * bw : (b + 1) * bw], ps[b], AF.Sigmoid,
                    bias=btile[:, j : j + 1],
                )
            hrtile = hrp.tile([128, b_shard], FP8, name="hrtile")
            nc.vector.tensor_mul(hrtile, hts[j], rtile)
            hrs.append(hrtile)

        # U+C fused per j, out chain in-place in ctile, chunked per bank so
        # the tail (ACT -> DVE -> store) pipelines at 512 granularity.
        for j in range(nh):
            ps = gate_matmuls(j, 2, xts, 3, hts)
            util = up.tile([128, b_shard], BF16, name="utile")
            for b in range(nb):
                nc.scalar.activation(
                    util[:, b * bw : (b + 1) * bw], ps[b], AF.Sigmoid,
                    bias=btile[:, nh + j : nh + j + 1],
                )
            ps = gate_matmuls(j, 4, xts, 5, hrs)
            ctile = cp.tile([128, b_shard], BF16, name="ctile")
            for b in range(nb):
                s = slice(b * bw, (b + 1) * bw)
                nc.scalar.activation(
                    ctile[:, s], ps[b], AF.Tanh,
                    bias=btile[:, 2 * nh + j : 2 * nh + j + 1],
                )
                # o = h + u*(c - h), computed in place in ctile (bf16)
                nc.vector.tensor_sub(ctile[:, s], ctile[:, s], hts[j][:, s])
                nc.vector.tensor_mul(ctile[:, s], util[:, s], ctile[:, s])
                nc.vector.tensor_add(ctile[:, s], ctile[:, s], hts[j][:, s])
                # per-bank store on the sync HWDGE ring (idle after loads):
                # pipelines the tail instead of one big end-of-gate store.
                nc.sync.dma_start(out[j * 128 : (j + 1) * 128, s], ctile[:, s])

    # Bacc lowering: splits multi-wait sync into InstEventSemaphore ops
    # (hardware allows one wait per instruction), allocates registers, etc.
    nc.compile()
    return nc


def pack_inputs(inputs, d=D, b_shard=B_SHARD, n_shards=N_CORES):
    """Host-side shard + transpose + cast. Returns per-shard input maps."""
    nk = d // 128
    nh = d // 128
    x = np.asarray(inputs["x_t"], np.float32)
    h = np.asarray(inputs["h_prev"], np.float32)

    mats = [inputs["W_r"], inputs["U_r"], inputs["W_u"], inputs["U_u"],
            inputs["W"], inputs["U"]]
    wts = np.empty((6, nh, 128, nk * 128), ml_dtypes.bfloat16)
    for i, m in enumerate(mats):
        mt = np.asarray(m, np.float32).T.astype(ml_dtypes.bfloat16)  # [in, out]
        # wts[i, j, p, k*128+m] = M.T[k*128+p, j*128+m]
        wts[i] = mt.reshape(nk, 128, nh, 128).transpose(2, 1, 0, 3).reshape(
            nh, 128, nk * 128
        )

    b_r = np.asarray(inputs["b_Wr"], np.float32) + np.asarray(inputs["b_Ur"], np.float32)
    b_u = np.asarray(inputs["b_Wu"], np.float32) + np.asarray(inputs["b_Uu"], np.float32)
    b_c = np.asarray(inputs["b_W"], np.float32) + np.asarray(inputs["b_U"], np.float32)
    bias = np.concatenate(
        [bb.reshape(nh, 128).T for bb in (b_r, b_u, b_c)], axis=1
    ).astype(np.float32)  # [128, 3*nh]

    in_maps = []
    for s in range(n_shards):
        rows = slice(s * b_shard, (s + 1) * b_shard)
        xT = np.ascontiguousarray(x[rows].T).astype(ml_dtypes.bfloat16)
        hT = np.ascontiguousarray(h[rows].T).astype(ml_dtypes.bfloat16)
        in_maps.append({"xt": xT, "ht": hT, "wts": wts, "bias": bias})
    return in_maps


_NC_CACHE = {}


def _get_nc():
    if "nc" not in _NC_CACHE:
        _NC_CACHE["nc"] = build_nc()
    return _NC_CACHE["nc"]


def _run(inputs, **spmd_kwargs):
    nc = _get_nc()
    in_maps = pack_inputs(inputs)
    res = run_bass_kernel_spmd(nc, in_maps, list(range(N_CORES)), **spmd_kwargs)
    out = np.empty((B, D), np.float32)
    for c in range(N_CORES):
        out[c * B_SHARD : (c + 1) * B_SHARD, :] = (
            res.results[c]["out"].astype(np.float32).T
        )
    return out, [res]


def kernel(**inputs):
    out, _ = _run(inputs)
    return out


# revision 14
# speedup vs baseline: 1.1391x; 1.1370x over previous
"""GRU cell kernel for Trainium2, 8-core data-parallel, single dispatch.

Strategy
--------
Data-parallel on batch across 8 cores; each core processes its full
2048-row shard in ONE NEFF dispatch.  All on-chip compute happens in
*transposed space* ([hidden, batch]) so every matmul contraction lands
on SBUF partitions with no on-device transposes:

    r^T = sigmoid(W_r @ x^T + U_r @ h^T + b_r)
    u^T = sigmoid(W_u @ x^T + U_u @ h^T + b_u)
    c^T = tanh   (W   @ x^T + U  @ (h.r)^T + b_c)
    o^T = h^T + u^T * (c^T - h^T)

Hybrid-precision contraction: k-tiles 0..5 of every 1024-long
contraction run in bf16 (one 128-contraction MM each); k-tiles 6..7
run as ONE fp8_e4m3 DoubleRow matmul (256-contraction, 2 weights per
PE cell) -> 7 matmuls instead of 8 per (matrix, gate, moving-slice),
an ~11% tensor-engine cut.  All weights are pre-scaled x64 on the
host (exact for bf16: power of two) so the fp8 k-tiles avoid the
e4m3 subnormal floor and both precisions accumulate in one PSUM group
at a common scale, undone by the activation's scale=1/64.  Measured
end-to-end rel err 1.4e-2 vs the 2e-2 budget (bf16-only was 7.5e-3).

The 2048-wide batch gives 4 moving slices of 512 per stationary
weight, keeping the tensor engine at its ~216 ns/MM back-to-back
cadence (weight-swap drain stalls amortize).

DMA conveyor: HWDGE rings dispatch ~1 DMA instruction per ~600 ns
regardless of size, so weights load as contiguous per-(matrix, gate)
slabs (host-side packed) on the Sync ring while x rides the Scalar
ring; the two conveyors run in parallel and the R phase consumes
k-tiles in arrival order.  Output stores go per-PSUM-bank on the Sync
ring (idle after loads), pipelining the tail.

SBUF (per partition, ~208 KiB usable): weights 84K + x 28K + h 36K +
hr 16K + pools ~20K.  hr is stored fp8 (moving operand only) and the
output chain runs bf16 in-place in the c tile.

DMA rules: descriptors encode exactly ONE sync wait, so no load DMA
may target a recycled tile slot (loads carry only queue-FIFO waits ->
every DMA'd tile gets its own slot).
"""

import sys

sys.path.insert(0, "/opt/trn_rl_repo")

import numpy as np
import ml_dtypes
from contextlib import ExitStack

import concourse.bass as bass
import concourse.bacc as bacc
import concourse.mybir as mybir
from concourse import tile
from concourse.bass_utils import run_bass_kernel_spmd

BF16 = mybir.dt.bfloat16
FP8 = mybir.dt.float8e4
F32 = mybir.dt.float32
AF = mybir.ActivationFunctionType
DR = mybir.MatmulPerfMode.DoubleRow

N_CORES = 8
B = 16384
D = 1024  # IN == H
B_SHARD = B // N_CORES  # 2048 rows per core, single dispatch
BW = 512  # matmul moving width (one fp32 PSUM bank)
NKB = 6  # k-tiles 0..NKB-1 in bf16; k-tiles NKB..7 via one fp8 DoubleRow MM
WSCALE = 64.0  # weight pre-scale (exact in bf16), undone in the activation


def build_nc(d=D, b_shard=B_SHARD, bw=BW):
    """Build the SPMD per-core Bass program.

    Packed weight order: 0=W_r, 1=U_r, 2=W_u, 3=U_u, 4=W, 5=U.
    Bias columns: [r: 0..nh) [u: nh..2nh) [c: 2nh..3nh).
    """
    nk = d // 128
    nh = d // 128
    nb = b_shard // bw

    nc = bacc.Bacc("TRN2", target_bir_lowering=False)
    xt = nc.dram_tensor("xt", [NKB * 128, b_shard], BF16, kind="ExternalInput")
    xt8 = nc.dram_tensor("xt8", [128, 2, b_shard], FP8, kind="ExternalInput")
    ht = nc.dram_tensor("ht", [d, b_shard], BF16, kind="ExternalInput")
    ht8 = nc.dram_tensor("ht8", [128, 2, b_shard], FP8, kind="ExternalInput")
    # bf16 weight slab (mat, j): [128, NKB*128]; fp8 pair tile: [128, 2, 128]
    wts = nc.dram_tensor("wts", [6, nh, 128, NKB * 128], BF16, kind="ExternalInput")
    wts8 = nc.dram_tensor("wts8", [6, nh, 128, 2, 128], FP8, kind="ExternalInput")
    bias = nc.dram_tensor("bias", [128, 3 * nh], F32, kind="ExternalInput")
    out = nc.dram_tensor("out", [d, b_shard], BF16, kind="ExternalOutput")

    with tile.TileContext(nc) as tc, ExitStack() as ctx:
        xp = ctx.enter_context(tc.tile_pool(name="xp", bufs=NKB))
        hp = ctx.enter_context(tc.tile_pool(name="hp", bufs=nk))
        hrp = ctx.enter_context(tc.tile_pool(name="hrp", bufs=NKB))
        hr8p = ctx.enter_context(tc.tile_pool(name="hr8p", bufs=1))
        rp = ctx.enter_context(tc.tile_pool(name="rp", bufs=2))
        up = ctx.enter_context(tc.tile_pool(name="up", bufs=2))
        cp = ctx.enter_context(tc.tile_pool(name="cp", bufs=3))
        # every weight slab gets its own slot: no DMA slot reuse anywhere
        # (pools size slots by their largest tile, so bf16 slabs and fp8
        # pair tiles get separate pools)
        wp = ctx.enter_context(tc.tile_pool(name="wp", bufs=6 * nh))
        w8p = ctx.enter_context(tc.tile_pool(name="w8p", bufs=6 * nh))
        bp = ctx.enter_context(tc.tile_pool(name="bp", bufs=1))
        pp = ctx.enter_context(tc.tile_pool(name="pp", bufs=8, space="PSUM"))

        wslabs, w8tiles = {}, {}

        def load_w(mat, j):
            t = wp.tile([128, NKB * 128], BF16, name="wslab")
            nc.sync.dma_start(t, wts[mat, j, :, :])
            wslabs[(mat, j)] = t
            t8 = w8p.tile([128, 2, 128], FP8, name="w8tile")
            nc.sync.dma_start(t8, wts8[mat, j, :, :, :])
            w8tiles[(mat, j)] = t8

        xts, hts = [None] * NKB, [None] * nk
        half = b_shard // 2

        def load_x(k):
            xts[k] = xp.tile([128, b_shard], BF16, name="xtile")
            nc.scalar.dma_start(xts[k][:, :half], xt[k * 128 : (k + 1) * 128, :half])
            nc.scalar.dma_start(xts[k][:, half:], xt[k * 128 : (k + 1) * 128, half:])

        def load_h(k):
            hts[k] = hp.tile([128, b_shard], BF16, name="htile")
            nc.sync.dma_start(hts[k][:, :half], ht[k * 128 : (k + 1) * 128, :half])
            nc.sync.dma_start(hts[k][:, half:], ht[k * 128 : (k + 1) * 128, half:])

        # scalar ring: x (bf16 then fp8 pair), then bias.  sync ring: gate-0
        # R slabs, h interleaved with later R slabs, then UC slabs in use
        # order.  The startup is HBM-bound; order mirrors consumption.
        for k in range(NKB):
            load_x(k)
        x8 = xp.tile([128, 2, b_shard], FP8, name="x8tile", bufs=1)
        nc.scalar.dma_start(x8, xt8[:, :, :])
        btile = bp.tile([128, 3 * nh], F32, name="btile")
        nc.scalar.dma_start(btile, bias[:, :])
        load_w(0, 0)
        load_w(1, 0)
        load_h(0)
        load_h(1)
        load_w(0, 1)
        load_w(1, 1)
        load_h(2)
        load_h(3)
        load_w(0, 2)
        load_w(1, 2)
        load_h(4)
        load_h(5)
        load_w(0, 3)
        load_w(1, 3)
        load_h(6)
        load_h(7)
        h8 = hp.tile([128, 2, b_shard], FP8, name="h8tile", bufs=1)
        nc.sync.dma_start(h8, ht8[:, :, :])
        for j in range(4, nh):
            load_w(0, j)
            load_w(1, j)
        for j in range(nh):
            for mat in (2, 3, 4, 5):
                load_w(mat, j)

        def gate_matmuls(j, mat_x, mat_h, mov_h, mov_h8):
            """Accumulate x-part + h-part for gate tile j into nb PSUM banks.

            Per matrix: bf16-slot k-tiles 0..NKB-1 (one MM each; the C
            gate's h-side tiles are actually fp8 hr, same slot role), then
            one fp8 DoubleRow MM covering the k-tile pair 6..7.
            """
            ps = [pp.tile([128, bw], F32, name="ps") for _ in range(nb)]
            for mi, (mat, movb, mov8) in enumerate(
                ((mat_x, xts, x8), (mat_h, mov_h, mov_h8))
            ):
                slab = wslabs[(mat, j)]
                for k in range(NKB):
                    lhsT = slab[:, k * 128 : (k + 1) * 128]
                    for b in range(nb):
                        nc.tensor.matmul(
                            ps[b],
                            lhsT,
                            movb[k][:, b * bw : (b + 1) * bw],
                            start=(mi == 0 and k == 0),
                            stop=False,
                        )
                w8 = w8tiles[(mat, j)]
                for b in range(nb):
                    nc.tensor.matmul(
                        ps[b],
                        w8[:, :, :],
                        mov8[:, :, b * bw : (b + 1) * bw],
                        start=False,
                        stop=(mi == 1),
                        perf_mode=DR,
                    )
            return ps

        inv = 1.0 / WSCALE

        # R phase: r = sigmoid(...); hr = h * r in fp8 (feeds the c matmuls).
        # Gates 0..NKB-1 fill plain fp8 tiles; gates 6,7 fill the pair tile.
        hrs = [None] * NKB
        hr8 = hr8p.tile([128, 2, b_shard], FP8, name="hr8tile")
        for j in range(nh):
            ps = gate_matmuls(j, 0, 1, hts, h8)
            rtile = rp.tile([128, b_shard], BF16, name="rtile")
            for b in range(nb):
                nc.scalar.activation(
                    rtile[:, b * bw : (b + 1) * bw], ps[b], AF.Sigmoid,
                    bias=btile[:, j : j + 1], scale=inv,
                )
            if j < NKB:
                hrs[j] = hrp.tile([128, b_shard], FP8, name="hrtile")
                nc.vector.tensor_mul(hrs[j], hts[j], rtile)
            else:
                nc.vector.tensor_mul(hr8[:, j - NKB, :], hts[j], rtile)

        # U+C fused per j, out chain in-place in ctile, chunked per bank so
        # the tail (ACT -> DVE -> store) pipelines at 512 granularity.
        for j in range(nh):
            ps = gate_matmuls(j, 2, 3, hts, h8)
            util = up.tile([128, b_shard], BF16, name="utile")
            for b in range(nb):
                nc.scalar.activation(
                    util[:, b * bw : (b + 1) * bw], ps[b], AF.Sigmoid,
                    bias=btile[:, nh + j : nh + j + 1], scale=inv,
                )
            ps = gate_matmuls(j, 4, 5, hrs, hr8)
            ctile = cp.tile([128, b_shard], BF16, name="ctile")
            for b in range(nb):
                s = slice(b * bw, (b + 1) * bw)
                nc.scalar.activation(
                    ctile[:, s], ps[b], AF.Tanh,
                    bias=btile[:, 2 * nh + j : 2 * nh + j + 1], scale=inv,
                )
                # o = h + u*(c - h), computed in place in ctile (bf16)
                nc.vector.tensor_sub(ctile[:, s], ctile[:, s], hts[j][:, s])
                nc.vector.tensor_mul(ctile[:, s], util[:, s], ctile[:, s])
                nc.vector.tensor_add(ctile[:, s], ctile[:, s], hts[j][:, s])
                # per-bank store on the sync HWDGE ring (idle after loads)
                nc.sync.dma_start(out[j * 128 : (j + 1) * 128, s], ctile[:, s])

    nc.compile()
    return nc


def pack_inputs(inputs, d=D, b_shard=B_SHARD, n_shards=N_CORES):
    """Host-side shard + transpose + cast. Returns per-shard input maps."""
    nk = d // 128
    nh = d // 128
    x = np.asarray(inputs["x_t"], np.float32)
    h = np.asarray(inputs["h_prev"], np.float32)
    split = NKB * 128

    mats = [inputs["W_r"], inputs["U_r"], inputs["W_u"], inputs["U_u"],
            inputs["W"], inputs["U"]]
    wts = np.empty((6, nh, 128, NKB * 128), ml_dtypes.bfloat16)
    wts8 = np.empty((6, nh, 128, 2, 128), ml_dtypes.float8_e4m3)
    for i, m in enumerate(mats):
        mt = WSCALE * np.asarray(m, np.float32).T  # [in, out], pre-scaled
        # bf16 part: wts[i, j, p, k*128+m] = 64*M.T[k*128+p, j*128+m], k<NKB
        wts[i] = (
            mt[:split]
            .astype(ml_dtypes.bfloat16)
            .reshape(NKB, 128, nh, 128)
            .transpose(2, 1, 0, 3)
            .reshape(nh, 128, NKB * 128)
        )
        # fp8 pair: wts8[i, j, p, t, m] = f8(64*M.T[split+t*128+p, j*128+m])
        wts8[i] = (
            mt[split:]
            .astype(ml_dtypes.float8_e4m3)
            .reshape(2, 128, nh, 128)
            .transpose(2, 1, 0, 3)
        )

    b_r = np.asarray(inputs["b_Wr"], np.float32) + np.asarray(inputs["b_Ur"], np.float32)
    b_u = np.asarray(inputs["b_Wu"], np.float32) + np.asarray(inputs["b_Uu"], np.float32)
    b_c = np.asarray(inputs["b_W"], np.float32) + np.asarray(inputs["b_U"], np.float32)
    bias = np.concatenate(
        [bb.reshape(nh, 128).T for bb in (b_r, b_u, b_c)], axis=1
    ).astype(np.float32)  # [128, 3*nh]

    in_maps = []
    for s in range(n_shards):
        rows = slice(s * b_shard, (s + 1) * b_shard)
        xT = np.ascontiguousarray(x[rows].T)
        hT = np.ascontiguousarray(h[rows].T)
        in_maps.append({
            "xt": xT[:split].astype(ml_dtypes.bfloat16),
            "xt8": np.ascontiguousarray(
                xT[split:].reshape(2, 128, b_shard).transpose(1, 0, 2)
            ).astype(ml_dtypes.float8_e4m3),
            "ht": hT.astype(ml_dtypes.bfloat16),
            "ht8": np.ascontiguousarray(
                hT[split:].reshape(2, 128, b_shard).transpose(1, 0, 2)
            ).astype(ml_dtypes.float8_e4m3),
            "wts": wts, "wts8": wts8, "bias": bias,
        })
    return in_maps


_NC_CACHE = {}


def _get_nc():
    if "nc" not in _NC_CACHE:
        _NC_CACHE["nc"] = build_nc()
    return _NC_CACHE["nc"]


def _run(inputs, **spmd_kwargs):
    nc = _get_nc()
    in_maps = pack_inputs(inputs)
    res = run_bass_kernel_spmd(nc, in_maps, list(range(N_CORES)), **spmd_kwargs)
    out = np.empty((B, D), np.float32)
    for c in range(N_CORES):
        out[c * B_SHARD : (c + 1) * B_SHARD, :] = (
            res.results[c]["out"].astype(np.float32).T
        )
    return out, [res]


def kernel(**inputs):
    out, _ = _run(inputs)
    return out


# revision 15
# speedup vs baseline: 1.3047x; 1.1454x over previous
"""GRU cell kernel for Trainium2, 8-core data-parallel, single dispatch.

Strategy
--------
Data-parallel on batch across 8 cores; each core processes its full
2048-row shard in ONE NEFF dispatch.  All on-chip compute happens in
*transposed space* ([hidden, batch]) so every matmul contraction lands
on SBUF partitions with no on-device transposes:

    r^T = sigmoid(W_r @ x^T + U_r @ h^T + b_r)
    u^T = sigmoid(W_u @ x^T + U_u @ h^T + b_u)
    c^T = tanh   (W   @ x^T + U  @ (h.r)^T + b_c)
    o^T = h^T + u^T * (c^T - h^T)

Hybrid-precision contraction (tensor-engine work cut ~25% vs bf16):
- W_r/W_u/U_u/W: k-tiles 0..5 bf16 (one 128-contraction MM each),
  k-tiles 6..7 as ONE fp8_e4m3 DoubleRow MM (2 weights per PE cell).
- U_r and U: FULLY fp8 -> their whole 1024-contraction runs as 4
  DoubleRow MMs.  Simulation shows this is free: the c-gate's moving
  operand (hr) is already fp8, and r-gate errors are doubly damped
  (sigmoid' <= 1/4, then averaged across the U contraction), so the
  end-to-end rel err stays 1.402e-2 (vs the 2e-2 budget; bf16-only
  was 7.5e-3).  The same split on U_u or W measures ~2e-2 and is NOT
  used.  All weights are pre-scaled x64 on the host (exact for bf16:
  power of two) so fp8 k-tiles clear the e4m3 subnormal floor and
  both precisions share one PSUM accumulation, undone by the
  activation's scale=1/64.

The 2048-wide batch gives 4 moving slices of 512 per stationary
weight, keeping the tensor engine at its ~216 ns/MM back-to-back
cadence (weight-swap drain stalls amortize).

DMA conveyor: HWDGE rings dispatch ~1 DMA instruction per ~600 ns
regardless of size, so weights load as contiguous per-(matrix, gate)
slabs (host-side packed) on the Sync ring while x rides the Scalar
ring; the two conveyors run in parallel and the R phase consumes
k-tiles in arrival order.  The R phase consumes h only in its fp8
pair form (2 MB instead of 4), shrinking the HBM-bound startup.
Output stores go per-PSUM-bank on the Sync ring (idle after loads),
pipelining the tail.

DMA rules: descriptors encode exactly ONE sync wait, so no load DMA
may target a recycled tile slot (loads carry only queue-FIFO waits ->
every DMA'd tile gets its own slot).
"""

import sys

sys.path.insert(0, "/opt/trn_rl_repo")

import numpy as np
import ml_dtypes
from contextlib import ExitStack

import concourse.bass as bass
import concourse.bacc as bacc
import concourse.mybir as mybir
from concourse import tile
from concourse.bass_utils import run_bass_kernel_spmd

BF16 = mybir.dt.bfloat16
FP8 = mybir.dt.float8e4
F32 = mybir.dt.float32
AF = mybir.ActivationFunctionType
DR = mybir.MatmulPerfMode.DoubleRow

N_CORES = 8
B = 16384
D = 1024  # IN == H
B_SHARD = B // N_CORES  # 2048 rows per core, single dispatch
BW = 512  # matmul moving width (one fp32 PSUM bank)
NKB = 6  # mixed matrices: k-tiles 0..NKB-1 bf16, NKB..7 one DoubleRow MM
WSCALE = 64.0  # weight pre-scale (exact in bf16), undone in the activation


def build_nc(d=D, b_shard=B_SHARD, bw=BW):
    """Build the SPMD per-core Bass program.

    Mixed-precision matrices (bf16 slab + fp8 pair): 0=W_r, 2=W_u,
    3=U_u, 4=W.  Fully-fp8 matrices (wts8f): q=0 -> U_r, q=1 -> U.
    Bias columns: [r: 0..nh) [u: nh..2nh) [c: 2nh..3nh).
    """
    nk = d // 128
    nh = d // 128
    nb = b_shard // bw
    npair = nk // 2

    nc = bacc.Bacc("TRN2", target_bir_lowering=False)
    xt = nc.dram_tensor("xt", [NKB * 128, b_shard], BF16, kind="ExternalInput")
    xt8 = nc.dram_tensor("xt8", [128, 2, b_shard], FP8, kind="ExternalInput")
    ht = nc.dram_tensor("ht", [d, b_shard], BF16, kind="ExternalInput")
    # full-fp8 h in pair-major layout: ht8f[p, t, i, col] = h^T[(2t+i)*128+p, col]
    ht8f = nc.dram_tensor("ht8f", [128, npair, 2, b_shard], FP8, kind="ExternalInput")
    wts = nc.dram_tensor("wts", [6, nh, 128, NKB * 128], BF16, kind="ExternalInput")
    wts8 = nc.dram_tensor("wts8", [6, nh, 128, 2, 128], FP8, kind="ExternalInput")
    # fully-fp8 weights (U_r, U): wts8f[q, j, p, kk, m]
    wts8f = nc.dram_tensor("wts8f", [2, nh, 128, nk, 128], FP8, kind="ExternalInput")
    bias = nc.dram_tensor("bias", [128, 3 * nh], F32, kind="ExternalInput")
    out = nc.dram_tensor("out", [d, b_shard], BF16, kind="ExternalOutput")

    with tile.TileContext(nc) as tc, ExitStack() as ctx:
        xp = ctx.enter_context(tc.tile_pool(name="xp", bufs=NKB))
        hp = ctx.enter_context(tc.tile_pool(name="hp", bufs=nk))
        h8p = ctx.enter_context(tc.tile_pool(name="h8p", bufs=1))
        hrp = ctx.enter_context(tc.tile_pool(name="hrp", bufs=npair))
        rp = ctx.enter_context(tc.tile_pool(name="rp", bufs=2))
        up = ctx.enter_context(tc.tile_pool(name="up", bufs=2))
        cp = ctx.enter_context(tc.tile_pool(name="cp", bufs=3))
        # every weight tile gets its own slot: no DMA slot reuse anywhere
        wp = ctx.enter_context(tc.tile_pool(name="wp", bufs=4 * nh))
        w8p = ctx.enter_context(tc.tile_pool(name="w8p", bufs=4 * nh))
        w8fp = ctx.enter_context(tc.tile_pool(name="w8fp", bufs=2 * nh))
        bp = ctx.enter_context(tc.tile_pool(name="bp", bufs=1))
        pp = ctx.enter_context(tc.tile_pool(name="pp", bufs=8, space="PSUM"))

        wslabs, w8tiles, w8f = {}, {}, {}

        def load_w(mat, j):
            t = wp.tile([128, NKB * 128], BF16, name="wslab")
            nc.sync.dma_start(t, wts[mat, j, :, :])
            wslabs[(mat, j)] = t
            t8 = w8p.tile([128, 2, 128], FP8, name="w8tile")
            nc.sync.dma_start(t8, wts8[mat, j, :, :, :])
            w8tiles[(mat, j)] = t8

        def load_w8f(q, j):
            t = w8fp.tile([128, nk, 128], FP8, name="w8ftile")
            nc.sync.dma_start(t, wts8f[q, j, :, :, :])
            w8f[(q, j)] = t

        xts, hts = [None] * NKB, [None] * nk
        half = b_shard // 2

        def load_x(k):
            xts[k] = xp.tile([128, b_shard], BF16, name="xtile")
            nc.scalar.dma_start(xts[k][:, :half], xt[k * 128 : (k + 1) * 128, :half])
            nc.scalar.dma_start(xts[k][:, half:], xt[k * 128 : (k + 1) * 128, half:])

        def load_h(k):
            hts[k] = hp.tile([128, b_shard], BF16, name="htile")
            nc.sync.dma_start(hts[k][:, :half], ht[k * 128 : (k + 1) * 128, :half])
            nc.sync.dma_start(hts[k][:, half:], ht[k * 128 : (k + 1) * 128, half:])

        # scalar ring: x bf16, x fp8 pair, bias.  sync ring: gate-0 R
        # weights, full-fp8 h (the R phase's only h need), bf16 h
        # interleaved with later R weights, then UC weights in use order.
        for k in range(NKB):
            load_x(k)
        x8 = xp.tile([128, 2, b_shard], FP8, name="x8tile", bufs=1)
        nc.scalar.dma_start(x8, xt8[:, :, :])
        btile = bp.tile([128, 3 * nh], F32, name="btile")
        nc.scalar.dma_start(btile, bias[:, :])
        load_w(0, 0)
        load_w8f(0, 0)
        h8 = h8p.tile([128, npair, 2, b_shard], FP8, name="h8tile")
        nc.sync.dma_start(h8[:, : npair // 2, :, :], ht8f[:, : npair // 2, :, :])
        nc.sync.dma_start(h8[:, npair // 2 :, :, :], ht8f[:, npair // 2 :, :, :])
        load_w(0, 1)
        load_w8f(0, 1)
        load_h(0)
        load_h(1)
        load_w(0, 2)
        load_w8f(0, 2)
        load_h(2)
        load_h(3)
        load_w(0, 3)
        load_w8f(0, 3)
        load_h(4)
        load_h(5)
        load_h(6)
        load_h(7)
        for j in range(4, nh):
            load_w(0, j)
            load_w8f(0, j)
        for j in range(nh):
            for mat in (2, 3):
                load_w(mat, j)
            load_w(4, j)
            load_w8f(1, j)

        def xpart(ps, mat, j):
            """x-side: 6 bf16 k-tile MMs + 1 DoubleRow pair MM (opens the
            accumulation group: start on each bank's first MM)."""
            slab = wslabs[(mat, j)]
            for k in range(NKB):
                lhsT = slab[:, k * 128 : (k + 1) * 128]
                for b in range(nb):
                    nc.tensor.matmul(
                        ps[b], lhsT, xts[k][:, b * bw : (b + 1) * bw],
                        start=(k == 0), stop=False,
                    )
            w8 = w8tiles[(mat, j)]
            for b in range(nb):
                nc.tensor.matmul(
                    ps[b], w8[:, :, :], x8[:, :, b * bw : (b + 1) * bw],
                    start=False, stop=False, perf_mode=DR,
                )

        def hpart_dr(ps, q, j, movpairs):
            """h-side, fully fp8: 4 DoubleRow MMs (closes the group)."""
            wt = w8f[(q, j)]
            for t in range(npair):
                lhsT = wt[:, 2 * t : 2 * t + 2, :]
                for b in range(nb):
                    nc.tensor.matmul(
                        ps[b], lhsT, movpairs[t][:, :, b * bw : (b + 1) * bw],
                        start=False, stop=(t == npair - 1), perf_mode=DR,
                    )

        def hpart_mixed(ps, mat, j):
            """h-side, mixed: 6 bf16 MMs + 1 DoubleRow (closes the group)."""
            slab = wslabs[(mat, j)]
            for k in range(NKB):
                lhsT = slab[:, k * 128 : (k + 1) * 128]
                for b in range(nb):
                    nc.tensor.matmul(
                        ps[b], lhsT, hts[k][:, b * bw : (b + 1) * bw],
                        start=False, stop=False,
                    )
            w8 = w8tiles[(mat, j)]
            for b in range(nb):
                nc.tensor.matmul(
                    ps[b], w8[:, :, :], h8[:, npair - 1, :, b * bw : (b + 1) * bw],
                    start=False, stop=True, perf_mode=DR,
                )

        inv = 1.0 / WSCALE
        h8pairs = [h8[:, t, :, :] for t in range(npair)]

        # R phase: r = sigmoid(W_r@x + U_r@h); hr = h * r in fp8 pair tiles
        hrpairs = [hrp.tile([128, 2, b_shard], FP8, name="hrtile") for _ in range(npair)]
        for j in range(nh):
            ps = [pp.tile([128, bw], F32, name="ps") for _ in range(nb)]
            xpart(ps, 0, j)
            hpart_dr(ps, 0, j, h8pairs)
            rtile = rp.tile([128, b_shard], BF16, name="rtile")
            for b in range(nb):
                nc.scalar.activation(
                    rtile[:, b * bw : (b + 1) * bw], ps[b], AF.Sigmoid,
                    bias=btile[:, j : j + 1], scale=inv,
                )
            nc.vector.tensor_mul(hrpairs[j // 2][:, j % 2, :], hts[j], rtile)

        # U+C fused per j, out chain in-place in ctile, chunked per bank so
        # the tail (ACT -> DVE -> store) pipelines at 512 granularity.
        for j in range(nh):
            ps = [pp.tile([128, bw], F32, name="ps") for _ in range(nb)]
            xpart(ps, 2, j)
            hpart_mixed(ps, 3, j)
            util = up.tile([128, b_shard], BF16, name="utile")
            for b in range(nb):
                nc.scalar.activation(
                    util[:, b * bw : (b + 1) * bw], ps[b], AF.Sigmoid,
                    bias=btile[:, nh + j : nh + j + 1], scale=inv,
                )
            ps = [pp.tile([128, bw], F32, name="ps") for _ in range(nb)]
            xpart(ps, 4, j)
            hpart_dr(ps, 1, j, hrpairs)
            ctile = cp.tile([128, b_shard], BF16, name="ctile")
            for b in range(nb):
                s = slice(b * bw, (b + 1) * bw)
                nc.scalar.activation(
                    ctile[:, s], ps[b], AF.Tanh,
                    bias=btile[:, 2 * nh + j : 2 * nh + j + 1], scale=inv,
                )
                # o = h + u*(c - h), computed in place in ctile (bf16)
                nc.vector.tensor_sub(ctile[:, s], ctile[:, s], hts[j][:, s])
                nc.vector.tensor_mul(ctile[:, s], util[:, s], ctile[:, s])
                nc.vector.tensor_add(ctile[:, s], ctile[:, s], hts[j][:, s])
                # per-bank store on the sync HWDGE ring (idle after loads)
                nc.sync.dma_start(out[j * 128 : (j + 1) * 128, s], ctile[:, s])

    nc.compile()
    return nc


def pack_inputs(inputs, d=D, b_shard=B_SHARD, n_shards=N_CORES):
    """Host-side shard + transpose + cast. Returns per-shard input maps."""
    nk = d // 128
    nh = d // 128
    npair = nk // 2
    x = np.asarray(inputs["x_t"], np.float32)
    h = np.asarray(inputs["h_prev"], np.float32)
    split = NKB * 128

    mats = [inputs["W_r"], inputs["U_r"], inputs["W_u"], inputs["U_u"],
            inputs["W"], inputs["U"]]
    wts = np.zeros((6, nh, 128, NKB * 128), ml_dtypes.bfloat16)
    wts8 = np.zeros((6, nh, 128, 2, 128), ml_dtypes.float8_e4m3)
    wts8f = np.empty((2, nh, 128, nk, 128), ml_dtypes.float8_e4m3)
    for i, m in enumerate(mats):
        mt = WSCALE * np.asarray(m, np.float32).T  # [in, out], pre-scaled
        if i in (1, 5):
            # fully-fp8: wts8f[q, j, p, kk, m'] = f8(64*M.T[kk*128+p, j*128+m'])
            q = 0 if i == 1 else 1
            wts8f[q] = (
                mt.astype(ml_dtypes.float8_e4m3)
                .reshape(nk, 128, nh, 128)
                .transpose(2, 1, 0, 3)
            )
            continue
        # bf16 slab: wts[i, j, p, k*128+m'] = bf16(64*M.T[k*128+p, j*128+m'])
        wts[i] = (
            mt[:split]
            .astype(ml_dtypes.bfloat16)
            .reshape(NKB, 128, nh, 128)
            .transpose(2, 1, 0, 3)
            .reshape(nh, 128, NKB * 128)
        )
        # fp8 pair: wts8[i, j, p, t, m'] = f8(64*M.T[split+t*128+p, j*128+m'])
        wts8[i] = (
            mt[split:]
            .astype(ml_dtypes.float8_e4m3)
            .reshape(2, 128, nh, 128)
            .transpose(2, 1, 0, 3)
        )

    b_r = np.asarray(inputs["b_Wr"], np.float32) + np.asarray(inputs["b_Ur"], np.float32)
    b_u = np.asarray(inputs["b_Wu"], np.float32) + np.asarray(inputs["b_Uu"], np.float32)
    b_c = np.asarray(inputs["b_W"], np.float32) + np.asarray(inputs["b_U"], np.float32)
    bias = np.concatenate(
        [bb.reshape(nh, 128).T for bb in (b_r, b_u, b_c)], axis=1
    ).astype(np.float32)  # [128, 3*nh]

    in_maps = []
    for s in range(n_shards):
        rows = slice(s * b_shard, (s + 1) * b_shard)
        xT = np.ascontiguousarray(x[rows].T)
        hT = np.ascontiguousarray(h[rows].T)
        in_maps.append({
            "xt": xT[:split].astype(ml_dtypes.bfloat16),
            "xt8": np.ascontiguousarray(
                xT[split:].reshape(2, 128, b_shard).transpose(1, 0, 2)
            ).astype(ml_dtypes.float8_e4m3),
            "ht": hT.astype(ml_dtypes.bfloat16),
            # ht8f[p, t, i, col] = f8(h^T[(2t+i)*128+p, col])
            "ht8f": np.ascontiguousarray(
                hT.reshape(npair, 2, 128, b_shard).transpose(2, 0, 1, 3)
            ).astype(ml_dtypes.float8_e4m3),
            "wts": wts, "wts8": wts8, "wts8f": wts8f, "bias": bias,
        })
    return in_maps


_NC_CACHE = {}


def _get_nc():
    if "nc" not in _NC_CACHE:
        _NC_CACHE["nc"] = build_nc()
    return _NC_CACHE["nc"]


def _run(inputs, **spmd_kwargs):
    nc = _get_nc()
    in_maps = pack_inputs(inputs)
    res = run_bass_kernel_spmd(nc, in_maps, list(range(N_CORES)), **spmd_kwargs)
    out = np.empty((B, D), np.float32)
    for c in range(N_CORES):
        out[c * B_SHARD : (c + 1) * B_SHARD, :] = (
            res.results[c]["out"].astype(np.float32).T
        )
    return out, [res]


def kernel(**inputs):
    out, _ = _run(inputs)
    return out


# revision 18
# speedup vs baseline: 1.3919x; 1.0668x over previous
"""GRU cell kernel for Trainium2, 8-core data-parallel, single dispatch.

Strategy
--------
Data-parallel on batch across 8 cores; each core processes its full
2048-row shard in ONE NEFF dispatch.  All on-chip compute happens in
*transposed space* ([hidden, batch]) so every matmul contraction lands
on SBUF partitions with no on-device transposes:

    r^T = sigmoid(W_r @ x^T + U_r @ h^T + b_r)
    u^T = sigmoid(W_u @ x^T + U_u @ h^T + b_u)
    c^T = tanh   (W   @ x^T + U  @ (h.r)^T + b_c)
    o^T = h^T + u^T * (c^T - h^T)

Hybrid-precision contraction (tensor-engine work cut ~25% vs bf16):
- W_r/W_u/U_u/W: k-tiles 0..5 bf16 (one 128-contraction MM each),
  k-tiles 6..7 as ONE fp8_e4m3 DoubleRow MM (2 weights per PE cell).
- U_r and U: FULLY fp8 -> their whole 1024-contraction runs as 4
  DoubleRow MMs.  Simulation shows this is free: the c-gate's moving
  operand (hr) is already fp8, and r-gate errors are doubly damped
  (sigmoid' <= 1/4, then averaged across the U contraction), so the
  end-to-end rel err stays 1.402e-2 (vs the 2e-2 budget; bf16-only
  was 7.5e-3).  The same split on U_u or W measures ~2e-2 and is NOT
  used.  All weights are pre-scaled x64 on the host (exact for bf16:
  power of two) so fp8 k-tiles clear the e4m3 subnormal floor and
  both precisions share one PSUM accumulation, undone by the
  activation's scale=1/64.

The 2048-wide batch gives 4 moving slices of 512 per stationary
weight, keeping the tensor engine at its ~216 ns/MM back-to-back
cadence (weight-swap drain stalls amortize).

DMA conveyor: HWDGE rings dispatch ~1 DMA instruction per ~600 ns
regardless of size, so weights load as contiguous per-(matrix, gate)
slabs (host-side packed) on the Sync ring while x rides the Scalar
ring; the two conveyors run in parallel and the R phase consumes
k-tiles in arrival order.  The R phase consumes h only in its fp8
pair form (2 MB instead of 4), shrinking the HBM-bound startup.
Output stores go per-PSUM-bank on the Sync ring (idle after loads),
pipelining the tail.

DMA rules: descriptors encode exactly ONE sync wait, so no load DMA
may target a recycled tile slot (loads carry only queue-FIFO waits ->
every DMA'd tile gets its own slot).
"""

import sys

sys.path.insert(0, "/opt/trn_rl_repo")

import numpy as np
import ml_dtypes
from contextlib import ExitStack

import concourse.bass as bass
import concourse.bacc as bacc
import concourse.mybir as mybir
from concourse import tile
from concourse.bass_utils import run_bass_kernel_spmd

BF16 = mybir.dt.bfloat16
FP8 = mybir.dt.float8e4
F32 = mybir.dt.float32
AF = mybir.ActivationFunctionType
DR = mybir.MatmulPerfMode.DoubleRow

N_CORES = 8
B = 16384
D = 1024  # IN == H
B_SHARD = B // N_CORES  # 2048 rows per core, single dispatch
BW = 512  # matmul moving width (one fp32 PSUM bank)
NKB = 6  # mixed matrices: k-tiles 0..NKB-1 bf16, NKB..7 one DoubleRow MM
WSCALE = 64.0  # weight pre-scale (exact in bf16), undone in the activation


def build_nc(d=D, b_shard=B_SHARD, bw=BW):
    """Build the SPMD per-core Bass program.

    Mixed-precision matrices (bf16 slab + fp8 pair): 0=W_r, 2=W_u,
    3=U_u, 4=W.  Fully-fp8 matrices (wts8f): q=0 -> U_r, q=1 -> U.
    Bias columns: [r: 0..nh) [u: nh..2nh) [c: 2nh..3nh).
    """
    nk = d // 128
    nh = d // 128
    nb = b_shard // bw
    npair = nk // 2

    nc = bacc.Bacc("TRN2", target_bir_lowering=False)
    xt = nc.dram_tensor("xt", [NKB * 128, b_shard], BF16, kind="ExternalInput")
    xt8f = nc.dram_tensor("xt8f", [128, npair, 2, b_shard], FP8, kind="ExternalInput")
    ht = nc.dram_tensor("ht", [d, b_shard], BF16, kind="ExternalInput")
    # full-fp8 h in pair-major layout: ht8f[p, t, i, col] = h^T[(2t+i)*128+p, col]
    ht8f = nc.dram_tensor("ht8f", [128, npair, 2, b_shard], FP8, kind="ExternalInput")
    wts = nc.dram_tensor("wts", [6, nh, 128, NKB * 128], BF16, kind="ExternalInput")
    wts8 = nc.dram_tensor("wts8", [6, nh, 128, 2, 128], FP8, kind="ExternalInput")
    # fully-fp8 weights: wts8f[q, j, p, kk, m], q: 0=U_r, 1=U, 2=W_r
    wts8f = nc.dram_tensor("wts8f", [3, nh, 128, nk, 128], FP8, kind="ExternalInput")
    bias = nc.dram_tensor("bias", [128, 3 * nh], F32, kind="ExternalInput")
    out = nc.dram_tensor("out", [d, b_shard], BF16, kind="ExternalOutput")

    with tile.TileContext(nc) as tc, ExitStack() as ctx:
        xp = ctx.enter_context(tc.tile_pool(name="xp", bufs=NKB))
        hp = ctx.enter_context(tc.tile_pool(name="hp", bufs=nk))
        h8p = ctx.enter_context(tc.tile_pool(name="h8p", bufs=1))
        hrp = ctx.enter_context(tc.tile_pool(name="hrp", bufs=npair))
        rp = ctx.enter_context(tc.tile_pool(name="rp", bufs=2))
        up = ctx.enter_context(tc.tile_pool(name="up", bufs=2))
        cp = ctx.enter_context(tc.tile_pool(name="cp", bufs=3))
        # every weight tile gets its own slot: no DMA slot reuse anywhere
        wp = ctx.enter_context(tc.tile_pool(name="wp", bufs=3 * nh))
        w8p = ctx.enter_context(tc.tile_pool(name="w8p", bufs=3 * nh))
        w8fp = ctx.enter_context(tc.tile_pool(name="w8fp", bufs=3 * nh))
        bp = ctx.enter_context(tc.tile_pool(name="bp", bufs=1))
        pp = ctx.enter_context(tc.tile_pool(name="pp", bufs=8, space="PSUM"))

        wslabs, w8tiles, w8f = {}, {}, {}

        def load_w(mat, j):
            t = wp.tile([128, NKB * 128], BF16, name="wslab")
            nc.sync.dma_start(t, wts[mat, j, :, :])
            wslabs[(mat, j)] = t
            t8 = w8p.tile([128, 2, 128], FP8, name="w8tile")
            nc.sync.dma_start(t8, wts8[mat, j, :, :, :])
            w8tiles[(mat, j)] = t8

        def load_w8f(q, j):
            t = w8fp.tile([128, nk, 128], FP8, name="w8ftile")
            nc.sync.dma_start(t, wts8f[q, j, :, :, :])
            w8f[(q, j)] = t

        xts, hts = [None] * NKB, [None] * nk
        half = b_shard // 2

        def load_x(k):
            xts[k] = xp.tile([128, b_shard], BF16, name="xtile")
            nc.scalar.dma_start(xts[k][:, :half], xt[k * 128 : (k + 1) * 128, :half])
            nc.scalar.dma_start(xts[k][:, half:], xt[k * 128 : (k + 1) * 128, half:])

        def load_h(k):
            hts[k] = hp.tile([128, b_shard], BF16, name="htile")
            nc.sync.dma_start(hts[k][:, :half], ht[k * 128 : (k + 1) * 128, :half])
            nc.sync.dma_start(hts[k][:, half:], ht[k * 128 : (k + 1) * 128, half:])

        # scalar ring: x bf16, x fp8 pair, bias.  sync ring: gate-0 R
        # weights, full-fp8 h (the R phase's only h need), bf16 h
        # interleaved with later R weights, then UC weights in use order.
        x8 = xp.tile([128, npair, 2, b_shard], FP8, name="x8tile", bufs=1)
        nc.scalar.dma_start(x8[:, : npair // 2, :, :], xt8f[:, : npair // 2, :, :])
        nc.scalar.dma_start(x8[:, npair // 2 :, :, :], xt8f[:, npair // 2 :, :, :])
        for k in range(NKB):
            load_x(k)
        btile = bp.tile([128, 3 * nh], F32, name="btile")
        nc.scalar.dma_start(btile, bias[:, :])
        load_w8f(2, 0)
        load_w8f(0, 0)
        h8 = h8p.tile([128, npair, 2, b_shard], FP8, name="h8tile")
        nc.sync.dma_start(h8[:, : npair // 2, :, :], ht8f[:, : npair // 2, :, :])
        nc.sync.dma_start(h8[:, npair // 2 :, :, :], ht8f[:, npair // 2 :, :, :])
        load_w8f(2, 1)
        load_w8f(0, 1)
        load_h(0)
        load_h(1)
        load_w8f(2, 2)
        load_w8f(0, 2)
        load_h(2)
        load_h(3)
        load_w8f(2, 3)
        load_w8f(0, 3)
        load_h(4)
        load_h(5)
        load_h(6)
        load_h(7)
        for j in range(4, nh):
            load_w8f(2, j)
            load_w8f(0, j)
        for j in range(nh):
            for mat in (2, 3):
                load_w(mat, j)
            load_w(4, j)
            load_w8f(1, j)

        def xpart(ps, mat, j):
            """x-side: 6 bf16 k-tile MMs + 1 DoubleRow pair MM (opens the
            accumulation group: start on each bank's first MM)."""
            slab = wslabs[(mat, j)]
            for k in range(NKB):
                lhsT = slab[:, k * 128 : (k + 1) * 128]
                for b in range(nb):
                    nc.tensor.matmul(
                        ps[b], lhsT, xts[k][:, b * bw : (b + 1) * bw],
                        start=(k == 0), stop=False,
                    )
            w8 = w8tiles[(mat, j)]
            for b in range(nb):
                nc.tensor.matmul(
                    ps[b], w8[:, :, :], x8[:, npair - 1, :, b * bw : (b + 1) * bw],
                    start=False, stop=False, perf_mode=DR,
                )

        def part_dr(ps, q, j, movpairs, open_group, close_group):
            """Fully-fp8 side: 4 DoubleRow MMs over the whole contraction."""
            wt = w8f[(q, j)]
            for t in range(npair):
                lhsT = wt[:, 2 * t : 2 * t + 2, :]
                for b in range(nb):
                    nc.tensor.matmul(
                        ps[b], lhsT, movpairs[t][:, :, b * bw : (b + 1) * bw],
                        start=(open_group and t == 0),
                        stop=(close_group and t == npair - 1), perf_mode=DR,
                    )

        def hpart_mixed(ps, mat, j):
            """h-side, mixed: 6 bf16 MMs + 1 DoubleRow (closes the group)."""
            slab = wslabs[(mat, j)]
            for k in range(NKB):
                lhsT = slab[:, k * 128 : (k + 1) * 128]
                for b in range(nb):
                    nc.tensor.matmul(
                        ps[b], lhsT, hts[k][:, b * bw : (b + 1) * bw],
                        start=False, stop=False,
                    )
            w8 = w8tiles[(mat, j)]
            for b in range(nb):
                nc.tensor.matmul(
                    ps[b], w8[:, :, :], h8[:, npair - 1, :, b * bw : (b + 1) * bw],
                    start=False, stop=True, perf_mode=DR,
                )

        inv = 1.0 / WSCALE
        h8pairs = [h8[:, t, :, :] for t in range(npair)]
        x8pairs = [x8[:, t, :, :] for t in range(npair)]

        # R phase: r = sigmoid(W_r@x + U_r@h); hr = h * r in fp8 pair tiles
        hrpairs = [hrp.tile([128, 2, b_shard], FP8, name="hrtile") for _ in range(npair)]
        for j in range(nh):
            ps = [pp.tile([128, bw], F32, name="ps") for _ in range(nb)]
            part_dr(ps, 2, j, x8pairs, True, False)
            part_dr(ps, 0, j, h8pairs, False, True)
            rtile = rp.tile([128, b_shard], BF16, name="rtile")
            for b in range(nb):
                nc.scalar.activation(
                    rtile[:, b * bw : (b + 1) * bw], ps[b], AF.Sigmoid,
                    bias=btile[:, j : j + 1], scale=inv,
                )
            nc.vector.tensor_mul(hrpairs[j // 2][:, j % 2, :], hts[j], rtile)

        # U+C fused per j, out chain in-place in ctile, chunked per bank so
        # the tail (ACT -> DVE -> store) pipelines at 512 granularity.
        for j in range(nh):
            ps = [pp.tile([128, bw], F32, name="ps") for _ in range(nb)]
            xpart(ps, 2, j)
            hpart_mixed(ps, 3, j)
            util = up.tile([128, b_shard], BF16, name="utile")
            for b in range(nb):
                nc.scalar.activation(
                    util[:, b * bw : (b + 1) * bw], ps[b], AF.Sigmoid,
                    bias=btile[:, nh + j : nh + j + 1], scale=inv,
                )
            ps = [pp.tile([128, bw], F32, name="ps") for _ in range(nb)]
            xpart(ps, 4, j)
            part_dr(ps, 1, j, hrpairs, False, True)
            ctile = cp.tile([128, b_shard], BF16, name="ctile")
            for b in range(nb):
                s = slice(b * bw, (b + 1) * bw)
                nc.scalar.activation(
                    ctile[:, s], ps[b], AF.Tanh,
                    bias=btile[:, 2 * nh + j : 2 * nh + j + 1], scale=inv,
                )
                # o = h + u*(c - h), computed in place in ctile (bf16)
                nc.vector.tensor_sub(ctile[:, s], ctile[:, s], hts[j][:, s])
                nc.vector.tensor_mul(ctile[:, s], util[:, s], ctile[:, s])
                nc.vector.tensor_add(ctile[:, s], ctile[:, s], hts[j][:, s])
                # per-bank store on the sync HWDGE ring (idle after loads)
                nc.sync.dma_start(out[j * 128 : (j + 1) * 128, s], ctile[:, s])

    nc.compile()
    return nc


def pack_inputs(inputs, d=D, b_shard=B_SHARD, n_shards=N_CORES):
    """Host-side shard + transpose + cast. Returns per-shard input maps."""
    nk = d // 128
    nh = d // 128
    npair = nk // 2
    x = np.asarray(inputs["x_t"], np.float32)
    h = np.asarray(inputs["h_prev"], np.float32)
    split = NKB * 128

    mats = [inputs["W_r"], inputs["U_r"], inputs["W_u"], inputs["U_u"],
            inputs["W"], inputs["U"]]
    wts = np.zeros((6, nh, 128, NKB * 128), ml_dtypes.bfloat16)
    wts8 = np.zeros((6, nh, 128, 2, 128), ml_dtypes.float8_e4m3)
    wts8f = np.empty((3, nh, 128, nk, 128), ml_dtypes.float8_e4m3)
    for i, m in enumerate(mats):
        mt = WSCALE * np.asarray(m, np.float32).T  # [in, out], pre-scaled
        if i in (0, 1, 5):
            # fully-fp8: wts8f[q, j, p, kk, m'] = f8(64*M.T[kk*128+p, j*128+m'])
            q = {1: 0, 5: 1, 0: 2}[i]
            wts8f[q] = (
                mt.astype(ml_dtypes.float8_e4m3)
                .reshape(nk, 128, nh, 128)
                .transpose(2, 1, 0, 3)
            )
            continue
        # bf16 slab: wts[i, j, p, k*128+m'] = bf16(64*M.T[k*128+p, j*128+m'])
        wts[i] = (
            mt[:split]
            .astype(ml_dtypes.bfloat16)
            .reshape(NKB, 128, nh, 128)
            .transpose(2, 1, 0, 3)
            .reshape(nh, 128, NKB * 128)
        )
        # fp8 pair: wts8[i, j, p, t, m'] = f8(64*M.T[split+t*128+p, j*128+m'])
        wts8[i] = (
            mt[split:]
            .astype(ml_dtypes.float8_e4m3)
            .reshape(2, 128, nh, 128)
            .transpose(2, 1, 0, 3)
        )

    b_r = np.asarray(inputs["b_Wr"], np.float32) + np.asarray(inputs["b_Ur"], np.float32)
    b_u = np.asarray(inputs["b_Wu"], np.float32) + np.asarray(inputs["b_Uu"], np.float32)
    b_c = np.asarray(inputs["b_W"], np.float32) + np.asarray(inputs["b_U"], np.float32)
    bias = np.concatenate(
        [bb.reshape(nh, 128).T for bb in (b_r, b_u, b_c)], axis=1
    ).astype(np.float32)  # [128, 3*nh]

    in_maps = []
    for s in range(n_shards):
        rows = slice(s * b_shard, (s + 1) * b_shard)
        xT = np.ascontiguousarray(x[rows].T)
        hT = np.ascontiguousarray(h[rows].T)
        in_maps.append({
            "xt": xT[:split].astype(ml_dtypes.bfloat16),
            "xt8f": np.ascontiguousarray(
                xT.reshape(npair, 2, 128, b_shard).transpose(2, 0, 1, 3)
            ).astype(ml_dtypes.float8_e4m3),
            "ht": hT.astype(ml_dtypes.bfloat16),
            # ht8f[p, t, i, col] = f8(h^T[(2t+i)*128+p, col])
            "ht8f": np.ascontiguousarray(
                hT.reshape(npair, 2, 128, b_shard).transpose(2, 0, 1, 3)
            ).astype(ml_dtypes.float8_e4m3),
            "wts": wts, "wts8": wts8, "wts8f": wts8f, "bias": bias,
        })
    return in_maps


_NC_CACHE = {}


def _get_nc():
    if "nc" not in _NC_CACHE:
        _NC_CACHE["nc"] = build_nc()
    return _NC_CACHE["nc"]


def _run(inputs, **spmd_kwargs):
    nc = _get_nc()
    in_maps = pack_inputs(inputs)
    res = run_bass_kernel_spmd(nc, in_maps, list(range(N_CORES)), **spmd_kwargs)
    out = np.empty((B, D), np.float32)
    for c in range(N_CORES):
        out[c * B_SHARD : (c + 1) * B_SHARD, :] = (
            res.results[c]["out"].astype(np.float32).T
        )
    return out, [res]


def kernel(**inputs):
    out, _ = _run(inputs)
    return out
